# revision 12
# baseline (speedup 1.0000x reference)
"""Distributed Trainium2 Bass kernel for blocked-sparse GNN message passing.

Computes  y = eps*diag*x + A @ (diag * mask * (A^T @ x)) * mask
where A is an NxN blocked-sparse matrix with per-edge 4x4 blocks.

Single-NEFF strategy (8 NeuronCores): edges are grouped by their col node
(scatter target of pass 1).  Crucially, w[col] = dm * (A^T x)[col] for every
edge in a chunk is produced BY that same chunk on the same core, so pass 2
(m_e = B_e @ w[col(e)]) needs no global barrier, no host gather, and no
second launch — and it reuses the SAME boo tile already in SBUF.

Host: relabel nodes sorted by col-degree, tile 128 nodes/tile, round-robin
tiles to cores, pad tiles to a shared degree schedule.  Pack per chunk:
boo [p, a, b, t, s] holding B[b,a] (bf16), xg [p, b, t, s] = x[row] (bf16),
dm table [p, (t a)] = diag*mask.

Device, per chunk:
  DVE  mult1: prod1[a,b,ts] = boo * bcast_a(xg)          (bf16 2x)
  PE   s,b-reduce via identity-weight accumulating matmuls -> psum1[(a t), r]
  DVE  jred: reduce residues -> acc[(a t)];  dmmul: w = acc * dm   (bf16)
  ACT  expand w over s (4 copies)                         -> w_exp[a, ts]
  DVE  mult2: prod2[a,b,ts] = boo * bcast_b(w_exp)        (bf16 2x)
  PE   a-reduce via identity matmuls -> psum2[(b ts)] = per-edge messages
  ACT  convert psum2 -> m_sb (bf16);  DMA m_sb -> mout

Host: map mout slots to edges, y = eps*diag*x + bincount(row, m)*mask.
"""

import sys
import numpy as np

sys.path.insert(0, "/opt/trn_rl_repo")


def _install_axon_profile_hook():
    """Provide antenv.axon_hooks (absent in this container) so
    run_bass_kernel_spmd(trace=True) can capture NTFF profiles."""
    import types
    if "antenv.axon_hooks" in sys.modules:
        return

    def get_axon_ntff_profile_hook():
        try:
            sys.path.insert(0, "/root/.axon_site")
            from trn_agent_boot.trn_boot import _ntff_profile_via_ctypes
            return _ntff_profile_via_ctypes("/opt/axon/libaxon_pjrt.so")
        except Exception:
            return None

    m = types.ModuleType("antenv.axon_hooks")
    m.get_axon_ntff_profile_hook = get_axon_ntff_profile_hook
    sys.modules["antenv.axon_hooks"] = m


_install_axon_profile_hook()

P = 128          # SBUF partitions
NCORES = 8
D = 4            # block dim
EPSILON = 0.01
SLOT_CAP = 384   # max per-partition slots per chunk (psum2 = 3 banks)
USE_BF16 = True


# ----------------------------------------------------------------------------
# Host-side planning
# ----------------------------------------------------------------------------

def _to_bf16(a):
    """Fast float32 -> bfloat16 (round-to-nearest-even), vectorized."""
    import ml_dtypes
    u = a.view(np.uint32)
    r = ((u >> 16) & 1) + 0x7FFF
    return ((u + r) >> 16).astype(np.uint16).view(ml_dtypes.bfloat16)


class PassPlan:
    """Static layout: edges grouped by dst node, nodes sorted by degree."""

    def __init__(self, dst, n_nodes):
        n_pad = -(-n_nodes // (P * NCORES)) * (P * NCORES)
        deg = np.bincount(dst, minlength=n_pad).astype(np.int64)
        order = np.argsort(-deg, kind="stable")     # node ids, degree desc
        pos = np.empty(n_pad, dtype=np.int64)
        pos[order] = np.arange(n_pad)
        n_tiles = n_pad // P
        self.tiles_per_core = n_tiles // NCORES
        tile_max = deg[order[::P]]                  # max degree of each tile
        dsch = np.maximum(tile_max[0::NCORES], 1)   # shared degree schedule
        self.deg_sched = dsch.astype(np.int64)
        self.slots_pp = int(self.deg_sched.sum())   # per-partition slots
        chunks = []
        k = 0
        K = self.tiles_per_core
        while k < K:
            d = int(self.deg_sched[k])
            t = 1
            while (k + t < K and self.deg_sched[k + t] == d
                   and (t + 1) * d <= SLOT_CAP):
                t += 1
            chunks.append((k, t, d))
            k += t
        self.chunks = chunks
        self.tile_base = np.concatenate([[0], np.cumsum(self.deg_sched)[:-1]])
        # per-edge coordinates
        q = pos[dst]
        r = q // P
        self.p = (q % P).astype(np.int64)           # partition
        self.c = (r % NCORES).astype(np.int64)      # core
        self.k = (r // NCORES).astype(np.int64)     # tile idx within core
        es = np.argsort(dst, kind="stable")
        cnt = np.bincount(dst, minlength=n_pad)
        starts = np.concatenate([[0], np.cumsum(cnt)[:-1]])
        s_sorted = np.arange(len(dst)) - starts[dst[es]]
        s = np.empty(len(dst), dtype=np.int64)
        s[es] = s_sorted
        self.s = s
        self.n_pad = n_pad
        self.pos = pos
        self.node_c = (pos // P) % NCORES
        self.node_k = (pos // P) // NCORES
        self.node_p = pos % P
        # per-edge chunk geometry: off0 (xg/m record base) and Td
        k0_of_k = np.zeros(self.tiles_per_core, dtype=np.int64)
        Td_of_k = np.zeros(self.tiles_per_core, dtype=np.int64)
        for (k0, T, d) in self.chunks:
            k0_of_k[k0:k0 + T] = k0
            Td_of_k[k0:k0 + T] = T * d
        k0e = k0_of_k[self.k]
        self.Tde = Td_of_k[self.k]
        self.ts = (self.k - k0e) * self.deg_sched[self.k] + self.s
        self.base_pp = self.tile_base[k0e]
        self.off0 = (4 * P * self.base_pp
                     + self.p * 4 * self.Tde + self.ts)

    def aux_table(self, values, tiles_per_core):
        """[128, K*4] table (p-major): aux[c][p*K*4 + k*4 + i] = values."""
        K = tiles_per_core
        out = np.zeros((NCORES, P * K * 4), dtype=np.float32)
        n_real = values.shape[0]
        c, k, p = self.node_c[:n_real], self.node_k[:n_real], self.node_p[:n_real]
        for i in range(4):
            out[c, p * (K * 4) + k * 4 + i] = values[:, i]
        return out


def plan_and_pack(x, edge_index, boo_values, mask, diag):
    N = x.shape[0]
    E = edge_index.shape[1]
    row = np.asarray(edge_index[0], dtype=np.int64)
    col = np.asarray(edge_index[1], dtype=np.int64)
    x = np.asarray(x, dtype=np.float32)
    boo = np.asarray(boo_values, dtype=np.float32)
    diag = np.asarray(diag, dtype=np.float32)
    mask = np.asarray(mask, dtype=np.float32)

    pl = PassPlan(col, N)
    K1, S1 = pl.tiles_per_core, pl.slots_pp

    # xg [p, b, t, s] = x[row][b]
    xg = np.zeros((NCORES, P * S1 * 4), dtype=np.float32)
    xr = x[row]
    for j in range(4):
        xg[pl.c, pl.off0 + j * pl.Tde] = xr[:, j]
    del xr
    xg = _to_bf16(xg) if USE_BF16 else xg

    # boo [p, a, b, t, s] = B[b, a]
    boo1 = np.zeros((NCORES, P * S1 * 16), dtype=np.float32)
    ebase0 = 16 * P * pl.base_pp + pl.p * 16 * pl.Tde + pl.ts
    CH = 1 << 19
    for lo in range(0, E, CH):
        hi = min(lo + CH, E)
        eb = ebase0[lo:hi]
        Td = pl.Tde[lo:hi]
        cc = pl.c[lo:hi]
        blk = boo[lo:hi]
        for a in range(4):
            for b in range(4):
                boo1[cc, eb + a * 4 * Td + b * Td] = blk[:, b, a]
    boo1 = _to_bf16(boo1) if USE_BF16 else boo1

    dmv = (diag * mask).astype(np.float32)
    dm = pl.aux_table(dmv, K1)

    meta = dict(N=N, E=E, K1=K1, S1=S1, chunks1=pl.chunks)
    in_maps = [{"boo1": boo1[c], "xg": xg[c], "dm": dm[c]}
               for c in range(NCORES)]
    post = dict(pl=pl, row=row, mask=mask, x=x, diag=diag)
    return meta, in_maps, post


# ----------------------------------------------------------------------------
# Device kernel
# ----------------------------------------------------------------------------

def build_kernel(meta):
    import concourse.bacc as bacc
    import concourse.tile as tile
    from concourse import mybir
    from concourse.bass import broadcast_tensor_aps, AP

    K1, S1 = meta["K1"], meta["S1"]
    f32 = mybir.dt.float32
    dt = mybir.dt.bfloat16 if USE_BF16 else f32
    nc = bacc.Bacc("TRN2", target_bir_lowering=False, debug=False,
                   num_devices=NCORES)
    boo1 = nc.dram_tensor("boo1", [P * S1 * 16], dt, kind="ExternalInput")
    xg = nc.dram_tensor("xg", [P * S1 * 4], dt, kind="ExternalInput")
    dm = nc.dram_tensor("dm", [P * K1 * 4], f32, kind="ExternalInput")
    ident = nc.dram_tensor("ident", [P * P], dt, kind="ExternalInput")
    mout = nc.dram_tensor("mout", [P * S1 * 4], dt, kind="ExternalOutput")

    with tile.TileContext(nc) as tc:
        with tc.tile_pool(name="sb", bufs=3) as pool, \
             tc.tile_pool(name="ps1", bufs=2, space="PSUM") as ps1_pool, \
             tc.tile_pool(name="ps2", bufs=2, space="PSUM") as ps2_pool, \
             tc.tile_pool(name="acc", bufs=1) as apool:
            ident_t = apool.tile([P, P], dt, tag="ident_t")
            nc.sync.dma_start(out=ident_t[:, :],
                              in_=ident.ap().rearrange("(p f) -> p f", p=P))
            dm_t = apool.tile([P, K1 * 4], f32, tag="dm_t")
            nc.sync.dma_start(out=dm_t[:, :],
                              in_=dm.ap().rearrange("(p f) -> p f", p=P))

            base = 0
            for (k0, T, d) in meta["chunks1"]:
                Td = T * d
                F4 = 4 * Td
                g1 = min(d, max(1, 512 // (4 * T)))
                nseg = -(-d // g1)
                boo_t = pool.tile([P, 4 * F4], dt, tag="boo_t")
                xg_t = pool.tile([P, F4], dt, tag="xg_t")
                prod1 = pool.tile([P, 4 * F4], dt, tag="prod1")
                prod2 = pool.tile([P, 4 * F4], dt, tag="prod2")
                w_t = pool.tile([P, 4 * T], dt, tag="w_t")
                acc_t = pool.tile([P, 4 * T], f32, tag="acc_t")
                w_exp = pool.tile([P, F4], dt, tag="w_exp")
                m_sb = pool.tile([P, F4], dt, tag="m_sb")
                ps1_t = ps1_pool.tile([P, 4 * T * g1], f32, tag="ps1")
                ps2_t = ps2_pool.tile([P, -(-F4 // 512) * 512], f32, tag="ps2")
                b0 = 16 * P * base
                x0 = 4 * P * base
                nc.sync.dma_start(
                    out=boo_t[:, :],
                    in_=boo1.ap()[b0:b0 + P * 4 * F4].rearrange(
                        "(p f) -> p f", p=P))
                nc.sync.dma_start(
                    out=xg_t[:, :],
                    in_=xg.ap()[x0:x0 + P * F4].rearrange("(p f) -> p f", p=P))
                # mult1: prod1[a,b,ts] = B[b,a] * x[b]
                in0 = boo_t[:, :].rearrange("p (a b ts) -> p a b ts",
                                            a=4, b=4, ts=Td)
                in1 = xg_t[:, :].rearrange("p (one b ts) -> p one b ts",
                                           one=1, b=4, ts=Td)
                in0b, in1b = broadcast_tensor_aps(in0, in1)
                nc.vector.tensor_tensor(
                    out=prod1[:, :].rearrange("p (a b ts) -> p a b ts",
                                              a=4, b=4, ts=Td),
                    in0=in0b, in1=in1b, op=mybir.AluOpType.mult)
                # PE1: accumulate over (b, s-groups) -> psum1[(a t), sres]
                pr = prod1[:, :]
                ps1a = ps1_t[:, :]
                for m in range(nseg):
                    s0 = m * g1
                    gs = min(g1, d - s0)
                    for b in range(4):
                        rhs = AP(pr.tensor, pr.offset + b * Td + s0,
                                 [pr.ap[0], [1, gs], [4 * Td, 4], [d, T]])
                        outp = AP(ps1a.tensor, ps1a.offset,
                                  [ps1a.ap[0], [1, gs], [T * g1, 4], [g1, T]])
                        nc.tensor.matmul(
                            out=outp, lhsT=ident_t[:, :], rhs=rhs,
                            start=(m == 0 and b == 0),
                            stop=(m == nseg - 1 and b == 3))
                # jred: acc[(a t)] = sum over residues
                nc.vector.tensor_reduce(
                    out=acc_t[:, :],
                    in_=ps1_t[:, :].rearrange("p (at sres) -> p at sres",
                                              at=4 * T, sres=g1),
                    axis=mybir.AxisListType.X,
                    op=mybir.AluOpType.add)
                # dmmul: w[(a t)] = acc * dm   (dm table is (t a)-major)
                nc.vector.tensor_tensor(
                    out=w_t[:, :].rearrange("p (a t) -> p a t", a=4),
                    in0=acc_t[:, :].rearrange("p (a t) -> p a t", a=4),
                    in1=dm_t[:, k0 * 4:(k0 + T) * 4].rearrange(
                        "p (t a) -> p a t", a=4),
                    op=mybir.AluOpType.mult)
                # ACT: expand w over s -> w_exp[a, (t s)]
                for a in range(4):
                    out3 = w_exp[:, a * Td:(a + 1) * Td].rearrange(
                        "p (t s) -> p t s", s=d)
                    in3 = w_t[:, a * T:(a + 1) * T].rearrange(
                        "p (t one) -> p t one", one=1)
                    _, in3b = broadcast_tensor_aps(out3, in3)
                    nc.scalar.activation(
                        out=out3, in_=in3b,
                        func=mybir.ActivationFunctionType.Copy)
                # mult2: prod2[a,b,ts] = B[b,a] * w[a]
                in1w = w_exp[:, :].rearrange("p (a one ts) -> p a one ts",
                                             a=4, one=1, ts=Td)
                in0c = boo_t[:, :].rearrange("p (a b ts) -> p a b ts",
                                             a=4, b=4, ts=Td)
                in0d, in1wb = broadcast_tensor_aps(in0c, in1w)
                nc.vector.tensor_tensor(
                    out=prod2[:, :].rearrange("p (a b ts) -> p a b ts",
                                              a=4, b=4, ts=Td),
                    in0=in0d, in1=in1wb, op=mybir.AluOpType.mult)
                # PE2: m[(b ts)] = sum_a prod2[a]
                for kk in range(-(-F4 // 512)):
                    wid = min(512, F4 - kk * 512)
                    for a in range(4):
                        nc.tensor.matmul(
                            out=ps2_t[:, kk * 512:kk * 512 + wid],
                            lhsT=ident_t[:, :],
                            rhs=prod2[:, a * F4 + kk * 512:
                                      a * F4 + kk * 512 + wid],
                            start=(a == 0), stop=(a == 3))
                # ACT: psum2 -> bf16 SBUF
                nc.scalar.activation(
                    out=m_sb[:, :], in_=ps2_t[:, :F4],
                    func=mybir.ActivationFunctionType.Copy)
                nc.sync.dma_start(
                    out=mout.ap()[x0:x0 + P * F4].rearrange(
                        "(p f) -> p f", p=P),
                    in_=m_sb[:, :])
                base += Td
    nc.compile()
    return nc


# ----------------------------------------------------------------------------
# Entry point
# ----------------------------------------------------------------------------

_COMPILED = {}
last_results = None
last_exec_ns = None


def kernel(x, edge_index, boo_values, mask, diag):
    global last_results, last_exec_ns
    meta, in_maps, post = plan_and_pack(
        np.asarray(x), np.asarray(edge_index), np.asarray(boo_values),
        np.asarray(mask), np.asarray(diag))

    key = (meta["K1"], meta["S1"], tuple(meta["chunks1"]))
    if key not in _COMPILED:
        _COMPILED[key] = build_kernel(meta)
    nc = _COMPILED[key]

    import concourse.bass_utils as _bu
    _bu.upload_artifacts = lambda tmpdir: ""   # no bucket in this container
    ident_np = np.eye(P, dtype=np.float32).reshape(-1)
    ident_np = _to_bf16(ident_np) if USE_BF16 else ident_np
    for im in in_maps:
        im["ident"] = ident_np
    res = _bu.run_bass_kernel_spmd(nc, in_maps, core_ids=list(range(NCORES)))
    last_results = (res,)
    last_exec_ns = res.exec_time_ns

    pl = post["pl"]
    N = meta["N"]
    mflat = np.stack([np.asarray(res.results[c]["mout"]).astype(np.float32)
                      for c in range(NCORES)])
    row, mask_, x_, diag_ = post["row"], post["mask"], post["x"], post["diag"]
    y = EPSILON * x_ * diag_
    for i in range(4):
        vals = mflat[pl.c, pl.off0 + i * pl.Tde]
        y[:, i] += (np.bincount(row, weights=vals, minlength=N)[:N]
                    * mask_[:, 0])
    return y.astype(np.float32)


# revision 15
# speedup vs baseline: 1.2617x; 1.2617x over previous
"""Distributed Trainium2 Bass kernel for blocked-sparse GNN message passing.

Computes  y = eps*diag*x + A @ (diag * mask * (A^T @ x)) * mask
where A is an NxN blocked-sparse matrix with per-edge 4x4 blocks.

Single-NEFF strategy (8 NeuronCores): edges are grouped by their col node
(scatter target of pass 1).  Crucially, w[col] = dm * (A^T x)[col] for every
edge in a chunk is produced BY that same chunk on the same core, so pass 2
(m_e = B_e @ w[col(e)]) needs no global barrier, no host gather, and no
second launch — and it reuses the SAME boo tile already in SBUF.

Host: relabel nodes sorted by col-degree, tile 128 nodes/tile, round-robin
tiles to cores, pad tiles to a shared degree schedule.  Pack per chunk:
boo [p, a, b, t, s] holding B[b,a] (bf16), xg [p, b, t, s] = x[row] (bf16),
dm table [p, (t a)] = diag*mask.

Device, per chunk:
  DVE  mult1: prod1[a,b,ts] = boo * bcast_a(xg)          (bf16 2x)
  PE   s,b-reduce via identity-weight accumulating matmuls -> psum1[(a t), r]
  DVE  jred: reduce residues -> acc[(a t)];  dmmul: w = acc * dm   (bf16)
  ACT  expand w over s (4 copies)                         -> w_exp[a, ts]
  DVE  mult2: prod2[a,b,ts] = boo * bcast_b(w_exp)        (bf16 2x)
  PE   a-reduce via identity matmuls -> psum2[(b ts)] = per-edge messages
  ACT  convert psum2 -> m_sb (bf16);  DMA m_sb -> mout

Host: map mout slots to edges, y = eps*diag*x + bincount(row, m)*mask.
"""

import sys
import numpy as np

sys.path.insert(0, "/opt/trn_rl_repo")


def _install_axon_profile_hook():
    """Provide antenv.axon_hooks (absent in this container) so
    run_bass_kernel_spmd(trace=True) can capture NTFF profiles."""
    import types
    if "antenv.axon_hooks" in sys.modules:
        return

    def get_axon_ntff_profile_hook():
        try:
            sys.path.insert(0, "/root/.axon_site")
            from trn_agent_boot.trn_boot import _ntff_profile_via_ctypes
            return _ntff_profile_via_ctypes("/opt/axon/libaxon_pjrt.so")
        except Exception:
            return None

    m = types.ModuleType("antenv.axon_hooks")
    m.get_axon_ntff_profile_hook = get_axon_ntff_profile_hook
    sys.modules["antenv.axon_hooks"] = m


_install_axon_profile_hook()

P = 128          # SBUF partitions
NCORES = 8
D = 4            # block dim
EPSILON = 0.01
SLOT_CAP = 384   # max per-partition slots per chunk (psum2 = 3 banks)
USE_BF16 = True


# ----------------------------------------------------------------------------
# Host-side planning
# ----------------------------------------------------------------------------

def _to_bf16(a):
    """Fast float32 -> bfloat16 (round-to-nearest-even), vectorized."""
    import ml_dtypes
    u = a.view(np.uint32)
    r = ((u >> 16) & 1) + 0x7FFF
    return ((u + r) >> 16).astype(np.uint16).view(ml_dtypes.bfloat16)


class PassPlan:
    """Static layout: edges grouped by dst node, nodes sorted by degree."""

    def __init__(self, dst, n_nodes):
        n_pad = -(-n_nodes // (P * NCORES)) * (P * NCORES)
        deg = np.bincount(dst, minlength=n_pad).astype(np.int64)
        order = np.argsort(-deg, kind="stable")     # node ids, degree desc
        pos = np.empty(n_pad, dtype=np.int64)
        pos[order] = np.arange(n_pad)
        n_tiles = n_pad // P
        self.tiles_per_core = n_tiles // NCORES
        tile_max = deg[order[::P]]                  # max degree of each tile
        dsch = np.maximum(tile_max[0::NCORES], 1)   # shared degree schedule
        self.deg_sched = dsch.astype(np.int64)
        self.slots_pp = int(self.deg_sched.sum())   # per-partition slots
        chunks = []
        k = 0
        K = self.tiles_per_core
        while k < K:
            d = int(self.deg_sched[k])
            t = 1
            while (k + t < K and self.deg_sched[k + t] == d
                   and (t + 1) * d <= SLOT_CAP):
                t += 1
            chunks.append((k, t, d))
            k += t
        self.chunks = chunks
        self.tile_base = np.concatenate([[0], np.cumsum(self.deg_sched)[:-1]])
        # per-edge coordinates
        q = pos[dst]
        r = q // P
        self.p = (q % P).astype(np.int64)           # partition
        self.c = (r % NCORES).astype(np.int64)      # core
        self.k = (r // NCORES).astype(np.int64)     # tile idx within core
        es = np.argsort(dst, kind="stable")
        cnt = np.bincount(dst, minlength=n_pad)
        starts = np.concatenate([[0], np.cumsum(cnt)[:-1]])
        s_sorted = np.arange(len(dst)) - starts[dst[es]]
        s = np.empty(len(dst), dtype=np.int64)
        s[es] = s_sorted
        self.s = s
        self.n_pad = n_pad
        self.pos = pos
        self.node_c = (pos // P) % NCORES
        self.node_k = (pos // P) // NCORES
        self.node_p = pos % P
        # per-edge chunk geometry: off0 (xg/m record base) and Td
        k0_of_k = np.zeros(self.tiles_per_core, dtype=np.int64)
        Td_of_k = np.zeros(self.tiles_per_core, dtype=np.int64)
        T_of_k = np.zeros(self.tiles_per_core, dtype=np.int64)
        for (k0, T, d) in self.chunks:
            k0_of_k[k0:k0 + T] = k0
            Td_of_k[k0:k0 + T] = T * d
            T_of_k[k0:k0 + T] = T
        k0e = k0_of_k[self.k]
        self.Tde = Td_of_k[self.k]
        # within-chunk slot index, t innermost: st = s*T + (k - k0)
        self.ts = self.s * T_of_k[self.k] + (self.k - k0e)
        self.base_pp = self.tile_base[k0e]
        self.off0 = (4 * P * self.base_pp
                     + self.p * 4 * self.Tde + self.ts)

    def aux_table(self, values, tiles_per_core):
        """[128, K*4] table (p-major): aux[c][p*K*4 + k*4 + i] = values."""
        K = tiles_per_core
        out = np.zeros((NCORES, P * K * 4), dtype=np.float32)
        n_real = values.shape[0]
        c, k, p = self.node_c[:n_real], self.node_k[:n_real], self.node_p[:n_real]
        for i in range(4):
            out[c, p * (K * 4) + k * 4 + i] = values[:, i]
        return out


def plan_and_pack(x, edge_index, boo_values, mask, diag):
    N = x.shape[0]
    E = edge_index.shape[1]
    row = np.asarray(edge_index[0], dtype=np.int64)
    col = np.asarray(edge_index[1], dtype=np.int64)
    x = np.asarray(x, dtype=np.float32)
    boo = np.asarray(boo_values, dtype=np.float32)
    diag = np.asarray(diag, dtype=np.float32)
    mask = np.asarray(mask, dtype=np.float32)

    pl = PassPlan(col, N)
    K1, S1 = pl.tiles_per_core, pl.slots_pp

    # xg [p, b, t, s] = x[row][b]
    xg = np.zeros((NCORES, P * S1 * 4), dtype=np.float32)
    xr = x[row]
    for j in range(4):
        xg[pl.c, pl.off0 + j * pl.Tde] = xr[:, j]
    del xr
    xg = _to_bf16(xg) if USE_BF16 else xg

    # boo [p, a, b, t, s] = B[b, a]
    boo1 = np.zeros((NCORES, P * S1 * 16), dtype=np.float32)
    ebase0 = 16 * P * pl.base_pp + pl.p * 16 * pl.Tde + pl.ts
    CH = 1 << 19
    for lo in range(0, E, CH):
        hi = min(lo + CH, E)
        eb = ebase0[lo:hi]
        Td = pl.Tde[lo:hi]
        cc = pl.c[lo:hi]
        blk = boo[lo:hi]
        for a in range(4):
            for b in range(4):
                boo1[cc, eb + a * 4 * Td + b * Td] = blk[:, b, a]
    boo1 = _to_bf16(boo1) if USE_BF16 else boo1

    dmv = (diag * mask).astype(np.float32)
    dm = pl.aux_table(dmv, K1)

    meta = dict(N=N, E=E, K1=K1, S1=S1, chunks1=pl.chunks)
    in_maps = [{"boo1": boo1[c], "xg": xg[c], "dm": dm[c]}
               for c in range(NCORES)]
    post = dict(pl=pl, row=row, mask=mask, x=x, diag=diag)
    return meta, in_maps, post


# ----------------------------------------------------------------------------
# Device kernel
# ----------------------------------------------------------------------------

def build_kernel(meta):
    import concourse.bacc as bacc
    import concourse.tile as tile
    from concourse import mybir
    from concourse.bass import broadcast_tensor_aps, AP

    K1, S1 = meta["K1"], meta["S1"]
    f32 = mybir.dt.float32
    dt = mybir.dt.bfloat16 if USE_BF16 else f32
    nc = bacc.Bacc("TRN2", target_bir_lowering=False, debug=False,
                   num_devices=NCORES)
    boo1 = nc.dram_tensor("boo1", [P * S1 * 16], dt, kind="ExternalInput")
    xg = nc.dram_tensor("xg", [P * S1 * 4], dt, kind="ExternalInput")
    dm = nc.dram_tensor("dm", [P * K1 * 4], f32, kind="ExternalInput")
    ident = nc.dram_tensor("ident", [P * P], dt, kind="ExternalInput")
    mout = nc.dram_tensor("mout", [P * S1 * 4], dt, kind="ExternalOutput")

    with tile.TileContext(nc) as tc:
        with tc.tile_pool(name="sb", bufs=3) as pool, \
             tc.tile_pool(name="ps1", bufs=2, space="PSUM") as ps1_pool, \
             tc.tile_pool(name="ps2", bufs=2, space="PSUM") as ps2_pool, \
             tc.tile_pool(name="acc", bufs=1) as apool:
            ident_t = apool.tile([P, P], dt, tag="ident_t")
            nc.sync.dma_start(out=ident_t[:, :],
                              in_=ident.ap().rearrange("(p f) -> p f", p=P))
            dm_t = apool.tile([P, K1 * 4], f32, tag="dm_t")
            nc.sync.dma_start(out=dm_t[:, :],
                              in_=dm.ap().rearrange("(p f) -> p f", p=P))

            base = 0
            for (k0, T, d) in meta["chunks1"]:
                Td = T * d
                F4 = 4 * Td
                g1 = min(d, max(1, 512 // (4 * T)))
                nseg = -(-d // g1)
                boo_t = pool.tile([P, 4 * F4], dt, tag="boo_t")
                xg_t = pool.tile([P, F4], dt, tag="xg_t")
                prod1 = pool.tile([P, 4 * F4], dt, tag="prod1")
                prod2 = pool.tile([P, 4 * F4], dt, tag="prod2")
                w_t = pool.tile([P, 4 * T], dt, tag="w_t")
                acc_t = pool.tile([P, 4 * T], f32, tag="acc_t")
                w_exp = pool.tile([P, F4], dt, tag="w_exp")
                m_sb = pool.tile([P, F4], dt, tag="m_sb")
                ps1_t = ps1_pool.tile([P, 4 * T * g1], f32, tag="ps1")
                ps2_t = ps2_pool.tile([P, -(-F4 // 512) * 512], f32, tag="ps2")
                b0 = 16 * P * base
                x0 = 4 * P * base
                nc.sync.dma_start(
                    out=boo_t[:, :],
                    in_=boo1.ap()[b0:b0 + P * 4 * F4].rearrange(
                        "(p f) -> p f", p=P))
                nc.sync.dma_start(
                    out=xg_t[:, :],
                    in_=xg.ap()[x0:x0 + P * F4].rearrange("(p f) -> p f", p=P))
                # mult1: prod1[a,b,ts] = B[b,a] * x[b]
                in0 = boo_t[:, :].rearrange("p (a b ts) -> p a b ts",
                                            a=4, b=4, ts=Td)
                in1 = xg_t[:, :].rearrange("p (one b ts) -> p one b ts",
                                           one=1, b=4, ts=Td)
                in0b, in1b = broadcast_tensor_aps(in0, in1)
                nc.vector.tensor_tensor(
                    out=prod1[:, :].rearrange("p (a b ts) -> p a b ts",
                                              a=4, b=4, ts=Td),
                    in0=in0b, in1=in1b, op=mybir.AluOpType.mult)
                # PE1: accumulate over (b, s-groups) -> psum1[a, sres, t]
                # layout is [a, b, s, t] with t innermost: contiguous T-runs
                pr = prod1[:, :]
                ps1a = ps1_t[:, :]
                for m in range(nseg):
                    s0 = m * g1
                    gs = min(g1, d - s0)
                    for b in range(4):
                        rhs = AP(pr.tensor, pr.offset + b * Td + s0 * T,
                                 [pr.ap[0], [4 * Td, 4], [T, gs], [1, T]])
                        outp = AP(ps1a.tensor, ps1a.offset,
                                  [ps1a.ap[0], [T * g1, 4], [T, gs], [1, T]])
                        nc.tensor.matmul(
                            out=outp, lhsT=ident_t[:, :], rhs=rhs,
                            start=(m == 0 and b == 0),
                            stop=(m == nseg - 1 and b == 3))
                # jred: acc[(a t)] = sum over sres (middle dim of psum1)
                nc.vector.tensor_reduce(
                    out=acc_t[:, :].rearrange("p (a t) -> p a t", a=4),
                    in_=AP(ps1a.tensor, ps1a.offset,
                           [ps1a.ap[0], [T * g1, 4], [1, T], [T, g1]]),
                    axis=mybir.AxisListType.X,
                    op=mybir.AluOpType.add)
                # dmmul: w[(a t)] = acc * dm   (dm table is (t a)-major)
                nc.vector.tensor_tensor(
                    out=w_t[:, :].rearrange("p (a t) -> p a t", a=4),
                    in0=acc_t[:, :].rearrange("p (a t) -> p a t", a=4),
                    in1=dm_t[:, k0 * 4:(k0 + T) * 4].rearrange(
                        "p (t a) -> p a t", a=4),
                    op=mybir.AluOpType.mult)
                # ACT: expand w over s -> w_exp[a, (s t)]
                for a in range(4):
                    out3 = w_exp[:, a * Td:(a + 1) * Td].rearrange(
                        "p (s t) -> p s t", s=d)
                    in3 = w_t[:, a * T:(a + 1) * T].rearrange(
                        "p (one t) -> p one t", one=1)
                    _, in3b = broadcast_tensor_aps(out3, in3)
                    nc.scalar.activation(
                        out=out3, in_=in3b,
                        func=mybir.ActivationFunctionType.Copy)
                # mult2: prod2[a,b,ts] = B[b,a] * w[a]
                in1w = w_exp[:, :].rearrange("p (a one ts) -> p a one ts",
                                             a=4, one=1, ts=Td)
                in0c = boo_t[:, :].rearrange("p (a b ts) -> p a b ts",
                                             a=4, b=4, ts=Td)
                in0d, in1wb = broadcast_tensor_aps(in0c, in1w)
                nc.vector.tensor_tensor(
                    out=prod2[:, :].rearrange("p (a b ts) -> p a b ts",
                                              a=4, b=4, ts=Td),
                    in0=in0d, in1=in1wb, op=mybir.AluOpType.mult)
                # PE2: m[(b ts)] = sum_a prod2[a]
                for kk in range(-(-F4 // 512)):
                    wid = min(512, F4 - kk * 512)
                    for a in range(4):
                        nc.tensor.matmul(
                            out=ps2_t[:, kk * 512:kk * 512 + wid],
                            lhsT=ident_t[:, :],
                            rhs=prod2[:, a * F4 + kk * 512:
                                      a * F4 + kk * 512 + wid],
                            start=(a == 0), stop=(a == 3))
                # ACT: psum2 -> bf16 SBUF
                nc.scalar.activation(
                    out=m_sb[:, :], in_=ps2_t[:, :F4],
                    func=mybir.ActivationFunctionType.Copy)
                nc.sync.dma_start(
                    out=mout.ap()[x0:x0 + P * F4].rearrange(
                        "(p f) -> p f", p=P),
                    in_=m_sb[:, :])
                base += Td
    nc.compile()
    return nc


# ----------------------------------------------------------------------------
# Entry point
# ----------------------------------------------------------------------------

_COMPILED = {}
last_results = None
last_exec_ns = None


def kernel(x, edge_index, boo_values, mask, diag):
    global last_results, last_exec_ns
    meta, in_maps, post = plan_and_pack(
        np.asarray(x), np.asarray(edge_index), np.asarray(boo_values),
        np.asarray(mask), np.asarray(diag))

    key = (meta["K1"], meta["S1"], tuple(meta["chunks1"]))
    if key not in _COMPILED:
        _COMPILED[key] = build_kernel(meta)
    nc = _COMPILED[key]

    import concourse.bass_utils as _bu
    _bu.upload_artifacts = lambda tmpdir: ""   # no bucket in this container
    ident_np = np.eye(P, dtype=np.float32).reshape(-1)
    ident_np = _to_bf16(ident_np) if USE_BF16 else ident_np
    for im in in_maps:
        im["ident"] = ident_np
    res = _bu.run_bass_kernel_spmd(nc, in_maps, core_ids=list(range(NCORES)))
    last_results = (res,)
    last_exec_ns = res.exec_time_ns

    pl = post["pl"]
    N = meta["N"]
    mflat = np.stack([np.asarray(res.results[c]["mout"]).astype(np.float32)
                      for c in range(NCORES)])
    row, mask_, x_, diag_ = post["row"], post["mask"], post["x"], post["diag"]
    y = EPSILON * x_ * diag_
    for i in range(4):
        vals = mflat[pl.c, pl.off0 + i * pl.Tde]
        y[:, i] += (np.bincount(row, weights=vals, minlength=N)[:N]
                    * mask_[:, 0])
    return y.astype(np.float32)


# revision 16
# speedup vs baseline: 1.2808x; 1.0151x over previous
"""Distributed Trainium2 Bass kernel for blocked-sparse GNN message passing.

Computes  y = eps*diag*x + A @ (diag * mask * (A^T @ x)) * mask
where A is an NxN blocked-sparse matrix with per-edge 4x4 blocks.

Single-NEFF strategy (8 NeuronCores): edges are grouped by their col node
(scatter target of pass 1).  Crucially, w[col] = dm * (A^T x)[col] for every
edge in a chunk is produced BY that same chunk on the same core, so pass 2
(m_e = B_e @ w[col(e)]) needs no global barrier, no host gather, and no
second launch — and it reuses the SAME boo tile already in SBUF.

Host: relabel nodes sorted by col-degree, tile 128 nodes/tile, round-robin
tiles to cores, pad tiles to a shared degree schedule.  Pack per chunk:
boo [p, a, b, t, s] holding B[b,a] (bf16), xg [p, b, t, s] = x[row] (bf16),
dm table [p, (t a)] = diag*mask.

Device, per chunk:
  DVE  mult1: prod1[a,b,ts] = boo * bcast_a(xg)          (bf16 2x)
  PE   s,b-reduce via identity-weight accumulating matmuls -> psum1[(a t), r]
  DVE  jred: reduce residues -> acc[(a t)];  dmmul: w = acc * dm   (bf16)
  ACT  expand w over s (4 copies)                         -> w_exp[a, ts]
  DVE  mult2: prod2[a,b,ts] = boo * bcast_b(w_exp)        (bf16 2x)
  PE   a-reduce via identity matmuls -> psum2[(b ts)] = per-edge messages
  ACT  convert psum2 -> m_sb (bf16);  DMA m_sb -> mout

Host: map mout slots to edges, y = eps*diag*x + bincount(row, m)*mask.
"""

import sys
import numpy as np

sys.path.insert(0, "/opt/trn_rl_repo")


def _install_axon_profile_hook():
    """Provide antenv.axon_hooks (absent in this container) so
    run_bass_kernel_spmd(trace=True) can capture NTFF profiles."""
    import types
    if "antenv.axon_hooks" in sys.modules:
        return

    def get_axon_ntff_profile_hook():
        try:
            sys.path.insert(0, "/root/.axon_site")
            from trn_agent_boot.trn_boot import _ntff_profile_via_ctypes
            return _ntff_profile_via_ctypes("/opt/axon/libaxon_pjrt.so")
        except Exception:
            return None

    m = types.ModuleType("antenv.axon_hooks")
    m.get_axon_ntff_profile_hook = get_axon_ntff_profile_hook
    sys.modules["antenv.axon_hooks"] = m


_install_axon_profile_hook()

P = 128          # SBUF partitions
NCORES = 8
D = 4            # block dim
EPSILON = 0.01
SLOT_CAP = 384   # max per-partition slots per chunk (psum2 = 3 banks)
USE_BF16 = True


# ----------------------------------------------------------------------------
# Host-side planning
# ----------------------------------------------------------------------------

def _to_bf16(a):
    """Fast float32 -> bfloat16 (round-to-nearest-even), vectorized."""
    import ml_dtypes
    u = a.view(np.uint32)
    r = ((u >> 16) & 1) + 0x7FFF
    return ((u + r) >> 16).astype(np.uint16).view(ml_dtypes.bfloat16)


class PassPlan:
    """Static layout: edges grouped by dst node, nodes sorted by degree."""

    def __init__(self, dst, n_nodes):
        n_pad = -(-n_nodes // (P * NCORES)) * (P * NCORES)
        deg = np.bincount(dst, minlength=n_pad).astype(np.int64)
        order = np.argsort(-deg, kind="stable")     # node ids, degree desc
        pos = np.empty(n_pad, dtype=np.int64)
        pos[order] = np.arange(n_pad)
        n_tiles = n_pad // P
        self.tiles_per_core = n_tiles // NCORES
        tile_max = deg[order[::P]]                  # max degree of each tile
        dsch = np.maximum(tile_max[0::NCORES], 1)   # shared degree schedule
        self.deg_sched = dsch.astype(np.int64)
        self.slots_pp = int(self.deg_sched.sum())   # per-partition slots
        chunks = []
        k = 0
        K = self.tiles_per_core
        while k < K:
            d = int(self.deg_sched[k])
            t = 1
            while (k + t < K and self.deg_sched[k + t] == d
                   and (t + 1) * d <= SLOT_CAP):
                t += 1
            chunks.append((k, t, d))
            k += t
        self.chunks = chunks
        self.tile_base = np.concatenate([[0], np.cumsum(self.deg_sched)[:-1]])
        # per-edge coordinates
        q = pos[dst]
        r = q // P
        self.p = (q % P).astype(np.int64)           # partition
        self.c = (r % NCORES).astype(np.int64)      # core
        self.k = (r // NCORES).astype(np.int64)     # tile idx within core
        es = np.argsort(dst, kind="stable")
        cnt = np.bincount(dst, minlength=n_pad)
        starts = np.concatenate([[0], np.cumsum(cnt)[:-1]])
        s_sorted = np.arange(len(dst)) - starts[dst[es]]
        s = np.empty(len(dst), dtype=np.int64)
        s[es] = s_sorted
        self.s = s
        self.n_pad = n_pad
        self.pos = pos
        self.node_c = (pos // P) % NCORES
        self.node_k = (pos // P) // NCORES
        self.node_p = pos % P
        # per-edge chunk geometry: off0 (xg/m record base) and Td
        k0_of_k = np.zeros(self.tiles_per_core, dtype=np.int64)
        Td_of_k = np.zeros(self.tiles_per_core, dtype=np.int64)
        T_of_k = np.zeros(self.tiles_per_core, dtype=np.int64)
        for (k0, T, d) in self.chunks:
            k0_of_k[k0:k0 + T] = k0
            Td_of_k[k0:k0 + T] = T * d
            T_of_k[k0:k0 + T] = T
        k0e = k0_of_k[self.k]
        self.Tde = Td_of_k[self.k]
        # within-chunk slot index, t innermost: st = s*T + (k - k0)
        self.ts = self.s * T_of_k[self.k] + (self.k - k0e)
        self.base_pp = self.tile_base[k0e]
        self.off0 = (4 * P * self.base_pp
                     + self.p * 4 * self.Tde + self.ts)

    def aux_table(self, values, tiles_per_core):
        """[128, K*4] table (p-major): aux[c][p*K*4 + k*4 + i] = values."""
        K = tiles_per_core
        out = np.zeros((NCORES, P * K * 4), dtype=np.float32)
        n_real = values.shape[0]
        c, k, p = self.node_c[:n_real], self.node_k[:n_real], self.node_p[:n_real]
        for i in range(4):
            out[c, p * (K * 4) + k * 4 + i] = values[:, i]
        return out


def plan_and_pack(x, edge_index, boo_values, mask, diag):
    N = x.shape[0]
    E = edge_index.shape[1]
    row = np.asarray(edge_index[0], dtype=np.int64)
    col = np.asarray(edge_index[1], dtype=np.int64)
    x = np.asarray(x, dtype=np.float32)
    boo = np.asarray(boo_values, dtype=np.float32)
    diag = np.asarray(diag, dtype=np.float32)
    mask = np.asarray(mask, dtype=np.float32)

    pl = PassPlan(col, N)
    K1, S1 = pl.tiles_per_core, pl.slots_pp

    # xg [p, b, t, s] = x[row][b]
    xg = np.zeros((NCORES, P * S1 * 4), dtype=np.float32)
    xr = x[row]
    for j in range(4):
        xg[pl.c, pl.off0 + j * pl.Tde] = xr[:, j]
    del xr
    xg = _to_bf16(xg) if USE_BF16 else xg

    # boo [p, a, b, t, s] = B[b, a]
    boo1 = np.zeros((NCORES, P * S1 * 16), dtype=np.float32)
    ebase0 = 16 * P * pl.base_pp + pl.p * 16 * pl.Tde + pl.ts
    CH = 1 << 19
    for lo in range(0, E, CH):
        hi = min(lo + CH, E)
        eb = ebase0[lo:hi]
        Td = pl.Tde[lo:hi]
        cc = pl.c[lo:hi]
        blk = boo[lo:hi]
        for a in range(4):
            for b in range(4):
                boo1[cc, eb + a * 4 * Td + b * Td] = blk[:, b, a]
    boo1 = _to_bf16(boo1) if USE_BF16 else boo1

    dmv = (diag * mask).astype(np.float32)
    dm = pl.aux_table(dmv, K1)

    meta = dict(N=N, E=E, K1=K1, S1=S1, chunks1=pl.chunks)
    in_maps = [{"boo1": boo1[c], "xg": xg[c], "dm": dm[c]}
               for c in range(NCORES)]
    post = dict(pl=pl, row=row, mask=mask, x=x, diag=diag)
    return meta, in_maps, post


# ----------------------------------------------------------------------------
# Device kernel
# ----------------------------------------------------------------------------

def build_kernel(meta):
    import concourse.bacc as bacc
    import concourse.tile as tile
    from concourse import mybir
    from concourse.bass import broadcast_tensor_aps, AP

    K1, S1 = meta["K1"], meta["S1"]
    f32 = mybir.dt.float32
    dt = mybir.dt.bfloat16 if USE_BF16 else f32
    nc = bacc.Bacc("TRN2", target_bir_lowering=False, debug=False,
                   num_devices=NCORES)
    boo1 = nc.dram_tensor("boo1", [P * S1 * 16], dt, kind="ExternalInput")
    xg = nc.dram_tensor("xg", [P * S1 * 4], dt, kind="ExternalInput")
    dm = nc.dram_tensor("dm", [P * K1 * 4], f32, kind="ExternalInput")
    ident = nc.dram_tensor("ident", [P * P], dt, kind="ExternalInput")
    mout = nc.dram_tensor("mout", [P * S1 * 4], dt, kind="ExternalOutput")

    with tile.TileContext(nc) as tc:
        with tc.tile_pool(name="sb", bufs=3) as pool, \
             tc.tile_pool(name="ps1", bufs=2, space="PSUM") as ps1_pool, \
             tc.tile_pool(name="ps2", bufs=2, space="PSUM") as ps2_pool, \
             tc.tile_pool(name="acc", bufs=1) as apool:
            ident_t = apool.tile([P, P], dt, tag="ident_t")
            nc.sync.dma_start(out=ident_t[:, :],
                              in_=ident.ap().rearrange("(p f) -> p f", p=P))
            dm_t = apool.tile([P, K1 * 4], f32, tag="dm_t")
            nc.sync.dma_start(out=dm_t[:, :],
                              in_=dm.ap().rearrange("(p f) -> p f", p=P))

            def emit_front(k0, T, d, base):
                """DMA in, mult1, PE1 -> returns ctx for the back half."""
                Td = T * d
                F4 = 4 * Td
                g1 = min(d, max(1, 512 // (4 * T)))
                nseg = -(-d // g1)
                boo_t = pool.tile([P, 4 * F4], dt, tag="boo_t")
                xg_t = pool.tile([P, F4], dt, tag="xg_t")
                prod1 = pool.tile([P, 4 * F4], dt, tag="prod1")
                ps1_t = ps1_pool.tile([P, 4 * T * g1], f32, tag="ps1")
                b0 = 16 * P * base
                x0 = 4 * P * base
                nc.sync.dma_start(
                    out=boo_t[:, :],
                    in_=boo1.ap()[b0:b0 + P * 4 * F4].rearrange(
                        "(p f) -> p f", p=P))
                nc.sync.dma_start(
                    out=xg_t[:, :],
                    in_=xg.ap()[x0:x0 + P * F4].rearrange("(p f) -> p f", p=P))
                # mult1: prod1[a,b,st] = B[b,a] * x[b]
                in0 = boo_t[:, :].rearrange("p (a b ts) -> p a b ts",
                                            a=4, b=4, ts=Td)
                in1 = xg_t[:, :].rearrange("p (one b ts) -> p one b ts",
                                           one=1, b=4, ts=Td)
                in0b, in1b = broadcast_tensor_aps(in0, in1)
                nc.vector.tensor_tensor(
                    out=prod1[:, :].rearrange("p (a b ts) -> p a b ts",
                                              a=4, b=4, ts=Td),
                    in0=in0b, in1=in1b, op=mybir.AluOpType.mult)
                # PE1: accumulate over (b, s-groups) -> psum1[a, sres, t]
                # layout is [a, b, s, t] with t innermost: contiguous T-runs
                pr = prod1[:, :]
                ps1a = ps1_t[:, :]
                for m in range(nseg):
                    s0 = m * g1
                    gs = min(g1, d - s0)
                    for b in range(4):
                        rhs = AP(pr.tensor, pr.offset + b * Td + s0 * T,
                                 [pr.ap[0], [4 * Td, 4], [T, gs], [1, T]])
                        outp = AP(ps1a.tensor, ps1a.offset,
                                  [ps1a.ap[0], [T * g1, 4], [T, gs], [1, T]])
                        nc.tensor.matmul(
                            out=outp, lhsT=ident_t[:, :], rhs=rhs,
                            start=(m == 0 and b == 0),
                            stop=(m == nseg - 1 and b == 3))
                return (k0, T, d, base, boo_t, ps1_t, g1)

            def emit_back(ctx):
                """jred, dmmul, ACT expand, mult2, PE2, convert, DMA out."""
                k0, T, d, base, boo_t, ps1_t, g1 = ctx
                Td = T * d
                F4 = 4 * Td
                x0 = 4 * P * base
                prod2 = pool.tile([P, 4 * F4], dt, tag="prod2")
                w_t = pool.tile([P, 4 * T], dt, tag="w_t")
                acc_t = pool.tile([P, 4 * T], f32, tag="acc_t")
                w_exp = pool.tile([P, F4], dt, tag="w_exp")
                m_sb = pool.tile([P, F4], dt, tag="m_sb")
                ps2_t = ps2_pool.tile([P, -(-F4 // 512) * 512], f32, tag="ps2")
                ps1a = ps1_t[:, :]
                # jred: acc[(a t)] = sum over sres (middle dim of psum1)
                nc.vector.tensor_reduce(
                    out=acc_t[:, :].rearrange("p (a t) -> p a t", a=4),
                    in_=AP(ps1a.tensor, ps1a.offset,
                           [ps1a.ap[0], [T * g1, 4], [1, T], [T, g1]]),
                    axis=mybir.AxisListType.X,
                    op=mybir.AluOpType.add)
                # dmmul: w[(a t)] = acc * dm   (dm table is (t a)-major)
                nc.vector.tensor_tensor(
                    out=w_t[:, :].rearrange("p (a t) -> p a t", a=4),
                    in0=acc_t[:, :].rearrange("p (a t) -> p a t", a=4),
                    in1=dm_t[:, k0 * 4:(k0 + T) * 4].rearrange(
                        "p (t a) -> p a t", a=4),
                    op=mybir.AluOpType.mult)
                # ACT: expand w over s -> w_exp[a, (s t)] (one instruction)
                out3 = w_exp[:, :].rearrange("p (a s t) -> p a s t", a=4, s=d)
                in3 = w_t[:, :].rearrange("p (a one t) -> p a one t",
                                          a=4, one=1)
                _, in3b = broadcast_tensor_aps(out3, in3)
                nc.scalar.activation(
                    out=out3, in_=in3b,
                    func=mybir.ActivationFunctionType.Copy)
                # mult2: prod2[a,b,st] = B[b,a] * w[a]
                in1w = w_exp[:, :].rearrange("p (a one ts) -> p a one ts",
                                             a=4, one=1, ts=Td)
                in0c = boo_t[:, :].rearrange("p (a b ts) -> p a b ts",
                                             a=4, b=4, ts=Td)
                in0d, in1wb = broadcast_tensor_aps(in0c, in1w)
                nc.vector.tensor_tensor(
                    out=prod2[:, :].rearrange("p (a b ts) -> p a b ts",
                                              a=4, b=4, ts=Td),
                    in0=in0d, in1=in1wb, op=mybir.AluOpType.mult)
                # PE2: m[(b st)] = sum_a prod2[a]
                for kk in range(-(-F4 // 512)):
                    wid = min(512, F4 - kk * 512)
                    for a in range(4):
                        nc.tensor.matmul(
                            out=ps2_t[:, kk * 512:kk * 512 + wid],
                            lhsT=ident_t[:, :],
                            rhs=prod2[:, a * F4 + kk * 512:
                                      a * F4 + kk * 512 + wid],
                            start=(a == 0), stop=(a == 3))
                # ACT: psum2 -> bf16 SBUF
                nc.scalar.activation(
                    out=m_sb[:, :], in_=ps2_t[:, :F4],
                    func=mybir.ActivationFunctionType.Copy)
                nc.sync.dma_start(
                    out=mout.ap()[x0:x0 + P * F4].rearrange(
                        "(p f) -> p f", p=P),
                    in_=m_sb[:, :])

            # software-pipelined emission: front(c) then back(c-1), so no
            # engine queues a back-half op behind its own chunk's producers
            base = 0
            pend = None
            for (k0, T, d) in meta["chunks1"]:
                ctx = emit_front(k0, T, d, base)
                if pend is not None:
                    emit_back(pend)
                pend = ctx
                base += T * d
            if pend is not None:
                emit_back(pend)
    nc.compile()
    return nc


# ----------------------------------------------------------------------------
# Entry point
# ----------------------------------------------------------------------------

_COMPILED = {}
last_results = None
last_exec_ns = None


def kernel(x, edge_index, boo_values, mask, diag):
    global last_results, last_exec_ns
    meta, in_maps, post = plan_and_pack(
        np.asarray(x), np.asarray(edge_index), np.asarray(boo_values),
        np.asarray(mask), np.asarray(diag))

    key = (meta["K1"], meta["S1"], tuple(meta["chunks1"]))
    if key not in _COMPILED:
        _COMPILED[key] = build_kernel(meta)
    nc = _COMPILED[key]

    import concourse.bass_utils as _bu
    _bu.upload_artifacts = lambda tmpdir: ""   # no bucket in this container
    ident_np = np.eye(P, dtype=np.float32).reshape(-1)
    ident_np = _to_bf16(ident_np) if USE_BF16 else ident_np
    for im in in_maps:
        im["ident"] = ident_np
    res = _bu.run_bass_kernel_spmd(nc, in_maps, core_ids=list(range(NCORES)))
    last_results = (res,)
    last_exec_ns = res.exec_time_ns

    pl = post["pl"]
    N = meta["N"]
    mflat = np.stack([np.asarray(res.results[c]["mout"]).astype(np.float32)
                      for c in range(NCORES)])
    row, mask_, x_, diag_ = post["row"], post["mask"], post["x"], post["diag"]
    y = EPSILON * x_ * diag_
    for i in range(4):
        vals = mflat[pl.c, pl.off0 + i * pl.Tde]
        y[:, i] += (np.bincount(row, weights=vals, minlength=N)[:N]
                    * mask_[:, 0])
    return y.astype(np.float32)


# revision 20
# speedup vs baseline: 1.4726x; 1.1498x over previous
"""Distributed Trainium2 Bass kernel for blocked-sparse GNN message passing.

Computes  y = eps*diag*x + A @ (diag * mask * (A^T @ x)) * mask
where A is an NxN blocked-sparse matrix with per-edge 4x4 blocks.

Single-NEFF strategy (8 NeuronCores): edges are grouped by their col node
(scatter target of pass 1).  Crucially, w[col] = dm * (A^T x)[col] for every
edge in a chunk is produced BY that same chunk on the same core, so pass 2
(m_e = B_e @ w[col(e)]) needs no global barrier, no host gather, and no
second launch — and it reuses the SAME boo tile already in SBUF.

Host: relabel nodes sorted by col-degree, tile 128 nodes/tile, round-robin
tiles to cores, pad tiles to a shared degree schedule.  Pack per chunk:
boo [p, a, b, t, s] holding B[b,a] (bf16), xg [p, b, t, s] = x[row] (bf16),
dm table [p, (t a)] = diag*mask.

Device, per chunk:
  DVE  mult1: prod1[a,b,ts] = boo * bcast_a(xg)          (bf16 2x)
  PE   s,b-reduce via identity-weight accumulating matmuls -> psum1[(a t), r]
  DVE  jred: reduce residues -> acc[(a t)];  dmmul: w = acc * dm   (bf16)
  ACT  expand w over s (4 copies)                         -> w_exp[a, ts]
  DVE  mult2: prod2[a,b,ts] = boo * bcast_b(w_exp)        (bf16 2x)
  PE   a-reduce via identity matmuls -> psum2[(b ts)] = per-edge messages
  ACT  convert psum2 -> m_sb (bf16);  DMA m_sb -> mout

Host: map mout slots to edges, y = eps*diag*x + bincount(row, m)*mask.
"""

import sys
import numpy as np

sys.path.insert(0, "/opt/trn_rl_repo")


def _install_axon_profile_hook():
    """Provide antenv.axon_hooks (absent in this container) so
    run_bass_kernel_spmd(trace=True) can capture NTFF profiles."""
    import types
    if "antenv.axon_hooks" in sys.modules:
        return

    def get_axon_ntff_profile_hook():
        try:
            sys.path.insert(0, "/root/.axon_site")
            from trn_agent_boot.trn_boot import _ntff_profile_via_ctypes
            return _ntff_profile_via_ctypes("/opt/axon/libaxon_pjrt.so")
        except Exception:
            return None

    m = types.ModuleType("antenv.axon_hooks")
    m.get_axon_ntff_profile_hook = get_axon_ntff_profile_hook
    sys.modules["antenv.axon_hooks"] = m


_install_axon_profile_hook()

P = 128          # SBUF partitions
NCORES = 8
D = 4            # block dim
EPSILON = 0.01
SLOT_CAP = 384   # max per-partition slots per chunk (psum2 = 3 banks)
USE_BF16 = True


# ----------------------------------------------------------------------------
# Host-side planning
# ----------------------------------------------------------------------------

def _to_bf16(a):
    """Fast float32 -> bfloat16 (round-to-nearest-even), vectorized."""
    import ml_dtypes
    u = a.view(np.uint32)
    r = ((u >> 16) & 1) + 0x7FFF
    return ((u + r) >> 16).astype(np.uint16).view(ml_dtypes.bfloat16)


class PassPlan:
    """Static layout: edges grouped by dst node, nodes sorted by degree."""

    def __init__(self, dst, n_nodes):
        n_pad = -(-n_nodes // (P * NCORES)) * (P * NCORES)
        deg = np.bincount(dst, minlength=n_pad).astype(np.int64)
        order = np.argsort(-deg, kind="stable")     # node ids, degree desc
        pos = np.empty(n_pad, dtype=np.int64)
        pos[order] = np.arange(n_pad)
        n_tiles = n_pad // P
        self.tiles_per_core = n_tiles // NCORES
        tile_max = deg[order[::P]]                  # max degree of each tile
        dsch = np.maximum(tile_max[0::NCORES], 1)   # shared degree schedule
        dsch = ((dsch + 1) // 2) * 2                # even: merges chunk tiles
        self.deg_sched = dsch.astype(np.int64)
        self.slots_pp = int(self.deg_sched.sum())   # per-partition slots
        chunks = []
        k = 0
        K = self.tiles_per_core
        while k < K:
            d = int(self.deg_sched[k])
            t = 1
            while (k + t < K and self.deg_sched[k + t] == d
                   and (t + 1) * d <= SLOT_CAP):
                t += 1
            chunks.append((k, t, d))
            k += t
        self.chunks = chunks
        self.tile_base = np.concatenate([[0], np.cumsum(self.deg_sched)[:-1]])
        # per-edge coordinates
        q = pos[dst]
        r = q // P
        self.p = (q % P).astype(np.int64)           # partition
        self.c = (r % NCORES).astype(np.int64)      # core
        self.k = (r // NCORES).astype(np.int64)     # tile idx within core
        es = np.argsort(dst, kind="stable")
        cnt = np.bincount(dst, minlength=n_pad)
        starts = np.concatenate([[0], np.cumsum(cnt)[:-1]])
        s_sorted = np.arange(len(dst)) - starts[dst[es]]
        s = np.empty(len(dst), dtype=np.int64)
        s[es] = s_sorted
        self.s = s
        self.n_pad = n_pad
        self.pos = pos
        self.node_c = (pos // P) % NCORES
        self.node_k = (pos // P) // NCORES
        self.node_p = pos % P
        # per-edge chunk geometry: off0 (xg/m record base) and Td
        k0_of_k = np.zeros(self.tiles_per_core, dtype=np.int64)
        Td_of_k = np.zeros(self.tiles_per_core, dtype=np.int64)
        T_of_k = np.zeros(self.tiles_per_core, dtype=np.int64)
        for (k0, T, d) in self.chunks:
            k0_of_k[k0:k0 + T] = k0
            Td_of_k[k0:k0 + T] = T * d
            T_of_k[k0:k0 + T] = T
        k0e = k0_of_k[self.k]
        self.Tde = Td_of_k[self.k]
        # within-chunk slot index, t innermost: st = s*T + (k - k0)
        self.ts = self.s * T_of_k[self.k] + (self.k - k0e)
        self.base_pp = self.tile_base[k0e]
        self.off0 = (4 * P * self.base_pp
                     + self.p * 4 * self.Tde + self.ts)

    def aux_table(self, values, tiles_per_core):
        """[128, K*4] table (p-major): aux[c][p*K*4 + k*4 + i] = values."""
        K = tiles_per_core
        out = np.zeros((NCORES, P * K * 4), dtype=np.float32)
        n_real = values.shape[0]
        c, k, p = self.node_c[:n_real], self.node_k[:n_real], self.node_p[:n_real]
        for i in range(4):
            out[c, p * (K * 4) + k * 4 + i] = values[:, i]
        return out


def plan_and_pack(x, edge_index, boo_values, mask, diag):
    N = x.shape[0]
    E = edge_index.shape[1]
    row = np.asarray(edge_index[0], dtype=np.int64)
    col = np.asarray(edge_index[1], dtype=np.int64)
    x = np.asarray(x, dtype=np.float32)
    boo = np.asarray(boo_values, dtype=np.float32)
    diag = np.asarray(diag, dtype=np.float32)
    mask = np.asarray(mask, dtype=np.float32)

    pl = PassPlan(col, N)
    K1, S1 = pl.tiles_per_core, pl.slots_pp

    # xg [p, b, t, s] = x[row][b]
    xg = np.zeros((NCORES, P * S1 * 4), dtype=np.float32)
    xr = x[row]
    for j in range(4):
        xg[pl.c, pl.off0 + j * pl.Tde] = xr[:, j]
    del xr
    xg = _to_bf16(xg) if USE_BF16 else xg

    # boo [p, a, b, t, s] = B[b, a]
    boo1 = np.zeros((NCORES, P * S1 * 16), dtype=np.float32)
    ebase0 = 16 * P * pl.base_pp + pl.p * 16 * pl.Tde + pl.ts
    CH = 1 << 19
    for lo in range(0, E, CH):
        hi = min(lo + CH, E)
        eb = ebase0[lo:hi]
        Td = pl.Tde[lo:hi]
        cc = pl.c[lo:hi]
        blk = boo[lo:hi]
        for a in range(4):
            for b in range(4):
                boo1[cc, eb + a * 4 * Td + b * Td] = blk[:, b, a]
    boo1 = _to_bf16(boo1) if USE_BF16 else boo1

    dmv = (diag * mask).astype(np.float32)
    dm = pl.aux_table(dmv, K1)

    meta = dict(N=N, E=E, K1=K1, S1=S1, chunks1=pl.chunks)
    in_maps = [{"boo1": boo1[c], "xg": xg[c], "dm": dm[c]}
               for c in range(NCORES)]
    post = dict(pl=pl, row=row, mask=mask, x=x, diag=diag)
    return meta, in_maps, post


# ----------------------------------------------------------------------------
# Device kernel
# ----------------------------------------------------------------------------

def build_kernel(meta):
    import concourse.bacc as bacc
    import concourse.tile as tile
    from concourse import mybir
    from concourse.bass import broadcast_tensor_aps, AP

    K1, S1 = meta["K1"], meta["S1"]
    f32 = mybir.dt.float32
    dt = mybir.dt.bfloat16 if USE_BF16 else f32
    nc = bacc.Bacc("TRN2", target_bir_lowering=False, debug=False,
                   num_devices=NCORES)
    boo1 = nc.dram_tensor("boo1", [P * S1 * 16], dt, kind="ExternalInput")
    xg = nc.dram_tensor("xg", [P * S1 * 4], dt, kind="ExternalInput")
    dm = nc.dram_tensor("dm", [P * K1 * 4], f32, kind="ExternalInput")
    ident = nc.dram_tensor("ident", [P * P], dt, kind="ExternalInput")
    mout = nc.dram_tensor("mout", [P * S1 * 4], dt, kind="ExternalOutput")

    with tile.TileContext(nc) as tc:
        with tc.tile_pool(name="sb", bufs=3) as pool, \
             tc.tile_pool(name="ps1", bufs=2, space="PSUM") as ps1_pool, \
             tc.tile_pool(name="ps2", bufs=2, space="PSUM") as ps2_pool, \
             tc.tile_pool(name="acc", bufs=1) as apool:
            ident_t = apool.tile([P, P], dt, tag="ident_t")
            nc.sync.dma_start(out=ident_t[:, :],
                              in_=ident.ap().rearrange("(p f) -> p f", p=P))
            dm_t = apool.tile([P, K1 * 4], f32, tag="dm_t")
            nc.sync.dma_start(out=dm_t[:, :],
                              in_=dm.ap().rearrange("(p f) -> p f", p=P))

            def emit_front(k0, T, d, base):
                """DMA in, mult1, PE1 -> returns ctx for the back half."""
                Td = T * d
                F4 = 4 * Td
                rT = min(Td, T * max(1, 128 // T))  # window run (mult of T)
                nw = -(-Td // rT)
                boo_t = pool.tile([P, 4 * F4], dt, tag="boo_t")
                xg_t = pool.tile([P, F4], dt, tag="xg_t")
                prod1 = pool.tile([P, 4 * F4], dt, tag="prod1")
                ps1_t = ps1_pool.tile([P, 4 * rT], f32, tag="ps1")
                b0 = 16 * P * base
                x0 = 4 * P * base
                nc.sync.dma_start(
                    out=boo_t[:, :],
                    in_=boo1.ap()[b0:b0 + P * 4 * F4].rearrange(
                        "(p f) -> p f", p=P))
                nc.sync.dma_start(
                    out=xg_t[:, :],
                    in_=xg.ap()[x0:x0 + P * F4].rearrange("(p f) -> p f", p=P))
                # mult1: prod1[a,b,st] = B[b,a] * x[b]
                in0 = boo_t[:, :].rearrange("p (a b ts) -> p a b ts",
                                            a=4, b=4, ts=Td)
                in1 = xg_t[:, :].rearrange("p (one b ts) -> p one b ts",
                                           one=1, b=4, ts=Td)
                in0b, in1b = broadcast_tensor_aps(in0, in1)
                nc.vector.tensor_tensor(
                    out=prod1[:, :].rearrange("p (a b ts) -> p a b ts",
                                              a=4, b=4, ts=Td),
                    in0=in0b, in1=in1b, op=mybir.AluOpType.mult)
                # PE1: window the flat st-axis in contiguous runs of rT;
                # window w accumulates into psum slot (st mod rT) -> full-
                # speed contiguous matmuls for every chunk shape
                pr = prod1[:, :]
                ps1a = ps1_t[:, :]
                for w in range(nw):
                    st0 = w * rT
                    run = min(rT, Td - st0)
                    for b in range(4):
                        rhs = AP(pr.tensor, pr.offset + b * Td + st0,
                                 [pr.ap[0], [4 * Td, 4], [1, run]])
                        outp = AP(ps1a.tensor, ps1a.offset,
                                  [ps1a.ap[0], [rT, 4], [1, run]])
                        nc.tensor.matmul(
                            out=outp, lhsT=ident_t[:, :], rhs=rhs,
                            start=(w == 0 and b == 0),
                            stop=(w == nw - 1 and b == 3))
                return (k0, T, d, base, boo_t, ps1_t, rT)

            def emit_back(ctx):
                """jred, dmmul, ACT expand, mult2, PE2, convert, DMA out."""
                k0, T, d, base, boo_t, ps1_t, rT = ctx
                Td = T * d
                F4 = 4 * Td
                x0 = 4 * P * base
                prod2 = pool.tile([P, 4 * F4], dt, tag="prod2")
                w_t = pool.tile([P, 4 * T], dt, tag="w_t")
                acc_t = pool.tile([P, 4 * T], f32, tag="acc_t")
                w_exp = pool.tile([P, F4], dt, tag="w_exp")
                m_sb = pool.tile([P, F4], dt, tag="m_sb")
                ps2_t = ps2_pool.tile([P, -(-F4 // 512) * 512], f32, tag="ps2")
                ps1a = ps1_t[:, :]
                # jred: acc[(a t)] = sum over s-residues of the psum windows
                nc.vector.tensor_reduce(
                    out=acc_t[:, :].rearrange("p (a t) -> p a t", a=4),
                    in_=AP(ps1a.tensor, ps1a.offset,
                           [ps1a.ap[0], [rT, 4], [1, T], [T, rT // T]]),
                    axis=mybir.AxisListType.X,
                    op=mybir.AluOpType.add)
                # dmmul: w[(a t)] = acc * dm   (dm table is (t a)-major)
                nc.vector.tensor_tensor(
                    out=w_t[:, :].rearrange("p (a t) -> p a t", a=4),
                    in0=acc_t[:, :].rearrange("p (a t) -> p a t", a=4),
                    in1=dm_t[:, k0 * 4:(k0 + T) * 4].rearrange(
                        "p (t a) -> p a t", a=4),
                    op=mybir.AluOpType.mult)
                # ACT: expand w over s -> w_exp[a, (s t)] (one instruction)
                out3 = w_exp[:, :].rearrange("p (a s t) -> p a s t", a=4, s=d)
                in3 = w_t[:, :].rearrange("p (a one t) -> p a one t",
                                          a=4, one=1)
                _, in3b = broadcast_tensor_aps(out3, in3)
                nc.scalar.activation(
                    out=out3, in_=in3b,
                    func=mybir.ActivationFunctionType.Copy)
                # mult2: prod2[a,b,st] = B[b,a] * w[a]
                in1w = w_exp[:, :].rearrange("p (a one ts) -> p a one ts",
                                             a=4, one=1, ts=Td)
                in0c = boo_t[:, :].rearrange("p (a b ts) -> p a b ts",
                                             a=4, b=4, ts=Td)
                in0d, in1wb = broadcast_tensor_aps(in0c, in1w)
                nc.vector.tensor_tensor(
                    out=prod2[:, :].rearrange("p (a b ts) -> p a b ts",
                                              a=4, b=4, ts=Td),
                    in0=in0d, in1=in1wb, op=mybir.AluOpType.mult)
                # PE2: m[(b st)] = sum_a prod2[a]
                for kk in range(-(-F4 // 512)):
                    wid = min(512, F4 - kk * 512)
                    for a in range(4):
                        nc.tensor.matmul(
                            out=ps2_t[:, kk * 512:kk * 512 + wid],
                            lhsT=ident_t[:, :],
                            rhs=prod2[:, a * F4 + kk * 512:
                                      a * F4 + kk * 512 + wid],
                            start=(a == 0), stop=(a == 3))
                # ACT: psum2 -> bf16 SBUF
                nc.scalar.activation(
                    out=m_sb[:, :], in_=ps2_t[:, :F4],
                    func=mybir.ActivationFunctionType.Copy)
                nc.sync.dma_start(
                    out=mout.ap()[x0:x0 + P * F4].rearrange(
                        "(p f) -> p f", p=P),
                    in_=m_sb[:, :])

            # software-pipelined emission: front(c) then back(c-1), so no
            # engine queues a back-half op behind its own chunk's producers
            base = 0
            pend = None
            for (k0, T, d) in meta["chunks1"]:
                ctx = emit_front(k0, T, d, base)
                if pend is not None:
                    emit_back(pend)
                pend = ctx
                base += T * d
            if pend is not None:
                emit_back(pend)
    nc.compile()
    return nc


# ----------------------------------------------------------------------------
# Entry point
# ----------------------------------------------------------------------------

_COMPILED = {}
last_results = None
last_exec_ns = None


def kernel(x, edge_index, boo_values, mask, diag):
    global last_results, last_exec_ns
    meta, in_maps, post = plan_and_pack(
        np.asarray(x), np.asarray(edge_index), np.asarray(boo_values),
        np.asarray(mask), np.asarray(diag))

    key = (meta["K1"], meta["S1"], tuple(meta["chunks1"]))
    if key not in _COMPILED:
        _COMPILED[key] = build_kernel(meta)
    nc = _COMPILED[key]

    import concourse.bass_utils as _bu
    _bu.upload_artifacts = lambda tmpdir: ""   # no bucket in this container
    ident_np = np.eye(P, dtype=np.float32).reshape(-1)
    ident_np = _to_bf16(ident_np) if USE_BF16 else ident_np
    for im in in_maps:
        im["ident"] = ident_np
    res = _bu.run_bass_kernel_spmd(nc, in_maps, core_ids=list(range(NCORES)))
    last_results = (res,)
    last_exec_ns = res.exec_time_ns

    pl = post["pl"]
    N = meta["N"]
    mflat = np.stack([np.asarray(res.results[c]["mout"]).astype(np.float32)
                      for c in range(NCORES)])
    row, mask_, x_, diag_ = post["row"], post["mask"], post["x"], post["diag"]
    y = EPSILON * x_ * diag_
    for i in range(4):
        vals = mflat[pl.c, pl.off0 + i * pl.Tde]
        y[:, i] += (np.bincount(row, weights=vals, minlength=N)[:N]
                    * mask_[:, 0])
    return y.astype(np.float32)
